# revision 79
# baseline (speedup 1.0000x reference)
"""Trainium2 Bass kernel for nn_CLLayer (SimCLR-style contrastive loss).

Math (reference, tau=0.5):
    h1 = elu(z1 @ W1.T + b1) @ W2.T + b2 ; h2 likewise
    n1, n2 = row-normalized h1, h2
    l1_i = log(sum_j exp(2*n1_i.n1_j) + sum_j exp(2*n1_i.n2_j) - e^2) - 2*n1_i.n2_i
    l2_i = log(sum_j exp(2*n2_i.n2_j) + sum_j exp(2*n2_j.n1_i... ) - e^2) - 2*...
    out = 0.5*(l1+l2)

Sharding: row-parallel over N=8192 (1024 rows/core, 8 cores).
Each core: projects its row block, normalizes, AllGathers normalized
embeddings, computes its row-strip of the three distinct similarity
products (S12, S22, S11), exp+row-sums on the fly, column-sums of
exp(2*S12) via a ReduceScatter (between2 = between.T so l2's "between"
row sums are column sums of S12's exp).  Only 3 of 4 N^2*D products run.

All matmuls are fp8e4 with perf_mode=DoubleRow (2x PE throughput;
each MM consumes a [P, 2, x] K-slab pair, K=256).  fp8 subnormals are
avoided by x16 pre-scales: weights are scaled x16 on the host (undone
via the activation `scale`), normalized embeddings x16 on device
(undone in the exp scale 2/256 and the positive-pair term -2/256).
Each AllGather is split into two column halves so pass A can start on
the first half while the second is still in flight.

Host-side prep: transposes z blocks / weights to K-major (PE wants K on
partitions), casts matmul operands to fp8e4 (ml_dtypes.float8_e4m3
matches TRN FP8_EXP4 bit-exactly below 240), and folds the ELU "-1"
into an adjusted fc2 bias (b2' = b2 - fc2_w.sum(1)) so ELU is computed
as relu(x) + min(exp(x),1) without the subtract (device ELU' = elu+1).
"""

import math
import os
from contextlib import ExitStack
from functools import lru_cache

import ml_dtypes
import numpy as np

import concourse.bacc as bacc
import concourse.bass as bass
import concourse.mybir as mybir
import concourse.tile as tile
from concourse.bass import make_scalar_value
from concourse.bass_utils import run_bass_kernel_spmd

N, D = 8192, 1024
NCORES = 8
BLK = N // NCORES  # 1024
P = 128
KO = D // P  # 8 k-tiles
NT = BLK // P  # 8 i-tiles per core
JP = NCORES  # 8 j-chunks of 1024 (= core blocks)
E2 = float(np.exp(2.0))  # exp(1/tau), tau=0.5
BF = mybir.dt.bfloat16
F8 = mybir.dt.float8e4
F32 = mybir.dt.float32
NS = 16.0  # fp8 pre-scale on normalized embeddings
WS = 16.0  # fp8 pre-scale on weights (host side)
DR = mybir.MatmulPerfMode.DoubleRow
AF = mybir.ActivationFunctionType
ALU = mybir.AluOpType


def _build():
    nc = bacc.Bacc("TRN2", target_bir_lowering=False, debug=False, num_devices=NCORES)

    # inputs arrive pre-arranged in on-chip layouts ([P, KO, x] K-major, [P, KO]
    # bias) — strided reads from host dram are latency-bound (a 4KB pt()-gather
    # measured 29us), so all reshaping happens on the host
    cid = nc.dram_tensor("cid", [1], mybir.dt.int32, kind="ExternalInput")
    z1t = nc.dram_tensor("z1t", [P, KO, BLK], F8, kind="ExternalInput")
    z2t = nc.dram_tensor("z2t", [P, KO, BLK], F8, kind="ExternalInput")
    w1t = nc.dram_tensor("w1t", [P, KO, D], F8, kind="ExternalInput")
    w2t = nc.dram_tensor("w2t", [P, KO, D], F8, kind="ExternalInput")
    b1 = nc.dram_tensor("b1", [P, KO], F32, kind="ExternalInput")
    b2p = nc.dram_tensor("b2p", [P, KO], F32, kind="ExternalInput")
    out = nc.dram_tensor("out", [BLK], F32, kind="ExternalOutput")

    kp = lambda ap: ap.rearrange("(ko ki) x -> ki ko x", ki=P)  # K-major -> [128, KO, x]
    pt = lambda ap: ap.rearrange("(t p) -> p t", p=P)  # [1024] -> [128, 8]

    rstack = ExitStack()
    with tile.TileContext(nc) as tc:
        with (
            tc.tile_pool(name="consts", bufs=1) as consts,
            tc.tile_pool(name="mats", bufs=1) as mats,
            tc.tile_pool(name="strip", bufs=1) as strip,
            tc.tile_pool(name="scratch", bufs=2) as scratch,
            tc.tile_pool(name="rhs", bufs=6) as rhsp,
            tc.tile_pool(name="expp", bufs=3) as expp,
            tc.tile_pool(name="small", bufs=1) as small,
            tc.tile_pool(name="psA", bufs=3, space="PSUM") as psA,
            tc.tile_pool(name="psB", bufs=2, space="PSUM") as psB,
            tc.tile_pool(name="dram", bufs=1, space="DRAM") as dram,
        ):
            # ---------------- constants / inputs (proj1's needs first) ----------------
            w1_sb = consts.tile([P, KO, D], F8)
            w2_sb = consts.tile([P, KO, D], F8)
            b1_sb = consts.tile([P, KO], F32)
            b2_sb = consts.tile([P, KO], F32)
            z_sb = mats.tile([P, KO, BLK], F8, tag="zt")
            # halved transfers so proj1's first matmuls start sooner
            nc.sync.dma_start(w1_sb[:, :, 0:512], w1t[:, :, 0:512])
            nc.sync.dma_start(z_sb[:, :, 0:512], z1t[:, :, 0:512])
            nc.sync.dma_start(w1_sb[:, :, 512:1024], w1t[:, :, 512:1024])
            nc.sync.dma_start(z_sb[:, :, 512:1024], z1t[:, :, 512:1024])
            nc.sync.dma_start(b1_sb[:], b1[:])
            nc.sync.dma_start(w2_sb[:], w2t[:])
            nc.sync.dma_start(b2_sb[:], b2p[:])
            # own tag: staging must not pin rhs-pool slots (WAR on the pool
            # rotation would stall pass A's prefetch until proj2-ch1 retires)
            z2a = rhsp.tile([P, KO, 512], F8, tag="zstage", name="z2a", bufs=2)
            z2b = rhsp.tile([P, KO, 512], F8, tag="zstage", name="z2b", bufs=2)
            nc.sync.dma_start(z2a[:], z2t[:, :, 0:512])
            nc.sync.dma_start(z2b[:], z2t[:, :, 512:1024])
            ones_bf = consts.tile([P, 1], BF)
            nc.vector.memset(ones_bf[:], 1.0)
            lnns = consts.tile([1, 1], F32)
            nc.vector.memset(lnns[:], float(math.log(NS)))

            n1_sb = mats.tile([P, KO, BLK], BF, tag="n1")
            n2_sb = mats.tile([P, KO, BLK], BF, tag="n2")

            ag_in = {}
            ag_out = {}
            for t in (1, 2):
                for h in (0, 1):
                    ag_in[t, h] = dram.tile([D, 512], F8, name=f"ag{t}{h}_in")
                    ag_out[t, h] = dram.tile(
                        [NCORES * D, 512], F8, addr_space="Shared", name=f"ag{t}{h}_out"
                    )
            rs_in = dram.tile([N], F32)
            rs_out = dram.tile([BLK], F32)
            rs11_in = dram.tile([N], F32)
            rs11_out = dram.tile([BLK], F32)
            rs22_in = dram.tile([N], F32)
            rs22_out = dram.tile([BLK], F32)
            rn_dram = dram.tile([2, BLK], BF)
            p_dram = dram.tile([BLK], F32)

            rg = [list(range(NCORES))]


            # ------------ projection + normalize (into n_sb + n_f8), per tensor ------------
            # Column-half-outer: each 512-column half runs L1 -> L2 -> sumsq ->
            # rn -> fp8 cast -> its AllGather trigger before the other half
            # starts, so AG h=0 is in flight ~half a projection early and
            # collective-duration variance hides under the remaining compute.
            def proj_l1(z_at, elu_sb, ch):
                # layer 1: a1T[o, i] = W1T.T @ zT (K=d);
                # elu+1 = relu(y) + min(exp(y), 1), y = ps/WS + b1
                sl = bass.ds(ch * 512, 512)
                for ot in range(KO):
                    ps = psA.tile([P, 512], F32, tag="ps_big", name="ps_l1")
                    for kt in range(0, KO, 2):
                        nc.tensor.matmul(
                            ps[:],
                            w1_sb[:, kt : kt + 2, bass.ts(ot, P)],
                            z_at(kt, ch),
                            start=(kt == 0),
                            stop=(kt == KO - 2),
                            perf_mode=DR,
                        )
                    bcol = b1_sb[:, ot : ot + 1]
                    e_t = scratch.tile([P, 512], F32, tag="e_t")
                    r_t = scratch.tile([P, 512], F32, tag="r_t")
                    nc.scalar.activation(e_t[:], ps[:], AF.Exp, bias=bcol, scale=1.0 / WS)
                    nc.scalar.activation(r_t[:], ps[:], AF.Relu, bias=bcol, scale=1.0 / WS)
                    nc.vector.tensor_scalar(e_t[:], e_t[:], 1.0, None, ALU.min)
                    nc.vector.tensor_tensor(elu_sb[:, ot, sl], e_t[:], r_t[:], ALU.add)

            def proj_l2_tail(elu_sb, n_sb, n_f8, rn_slot, t, ch):
                sl = bass.ds(ch * 512, 512)
                # layer 2 -> n_sb (holds hT until scaled)
                ssps = psB.tile([1, 512], F32, name=f"ssps{t}{ch}", tag="ps_small")
                for ot in range(KO):
                    ps = psA.tile([P, 512], F32, tag="ps_big", name="ps_l2")
                    for kt in range(0, KO, 2):
                        nc.tensor.matmul(
                            ps[:],
                            w2_sb[:, kt : kt + 2, bass.ts(ot, P)],
                            elu_sb[:, kt : kt + 2, sl],
                            start=(kt == 0),
                            stop=(kt == KO - 2),
                            perf_mode=DR,
                        )
                    # bias-add stays on DVE: it is also what drains L2's PSUM,
                    # and the scalar queue backlogs during its fetch-stall
                    # recovery (an Identity-ACT variant measured ~20us slower)
                    nc.vector.tensor_scalar(
                        n_sb[:, ot, sl], ps[:], 1.0 / WS, b2_sb[:, ot : ot + 1],
                        ALU.mult, ALU.add,
                    )
                    # sumsq over d (partitions) via ones-matmul on h*h; the
                    # square runs on DVE — the scalar queue is the proj-region
                    # bottleneck (late instruction-fetch wake + ACT backlog) and
                    # this also avoids Square<->Exp ACT-table reloads
                    sq = scratch.tile([P, 512], BF, tag="sq")
                    nc.vector.tensor_tensor(sq[:], n_sb[:, ot, sl], n_sb[:, ot, sl], ALU.mult)
                    nc.tensor.matmul(
                        ssps[:], ones_bf[:], sq[:],
                        start=(ot == 0), stop=(ot == KO - 1),
                    )
                # rn = NS/||h||: rsqrt = NS*Exp(-0.5*Ln(s)) on the ACT
                # tables (DVE reciprocal is single-lane slow; the Ln/Exp
                # tables already bound the kernel's overall accuracy)
                l_c = small.tile([1, 512], F32, tag="l_c", name=f"l_c{t}{ch}", bufs=2)
                rn_c = small.tile([1, 512], BF, tag="rn_c", name=f"rn_c{t}{ch}", bufs=2)
                nc.scalar.activation(l_c[:], ssps[:], AF.Ln)
                nc.scalar.activation(rn_c[:], l_c[:], AF.Exp, scale=-0.5, bias=lnns[:])
                nc.scalar.dma_start(rn_dram[rn_slot : rn_slot + 1, sl], rn_c[:])
                rn_bc = scratch.tile([P, 512], BF, tag="rnbc", bufs=2, name=f"rn_bc{t}{ch}")
                nc.scalar.dma_start(
                    rn_bc[:],
                    rn_dram[rn_slot : rn_slot + 1, sl].to_broadcast((P, 512)),
                )
                for kt in range(KO):
                    nc.vector.tensor_tensor(
                        n_f8[:, kt, sl], n_sb[:, kt, sl], rn_bc[:], ALU.mult
                    )
                nc.scalar.dma_start(kp(ag_in[t, ch][:]), n_f8[:, :, sl])
                nc.gpsimd.collective_compute(
                    "AllGather", ALU.bypass, replica_groups=rg,
                    ins=[ag_in[t, ch][:].opt()], outs=[ag_out[t, ch][:].opt()],
                )

            elu1 = mats.tile([P, KO, BLK], F8, tag="elu")
            # own slots: n_f8 ch-0 writes must not WAR-wait on the elu/z slots'
            # ch-1 readers, or the early AllGather trigger serializes away
            n1_f8 = mats.tile([P, KO, BLK], F8, tag="n1f8", name="n1_f8")
            elu2 = mats.tile([P, KO, BLK], F8, tag="elu2", name="elu2")
            n2_f8 = mats.tile([P, KO, BLK], F8, tag="n2f8", name="n2_f8")
            z1_at = lambda kt, ch: z_sb[:, kt : kt + 2, bass.ds(ch * 512, 512)]
            z2_at = lambda kt, ch: (z2a if ch == 0 else z2b)[:, kt : kt + 2, :]
            # phase-sequential: each (tensor, half) runs L1 then L2+tail so its
            # AllGather triggers as early as possible (AG1a ~60us); the next
            # phase's L1 matmuls then cover the tail's activation drain
            for ch in (0, 1):
                proj_l1(z1_at, elu1, ch)
                proj_l2_tail(elu1, n1_sb, n1_f8, 0, 1, ch)
                proj_l1(z2_at, elu2, ch)
                proj_l2_tail(elu2, n2_sb, n2_f8, 1, 2, ch)


            # rowsum partials: S11 (5 symmetric pair-cols) and S12 (8) share one
            # tile so a single reduce yields r11_local + r12
            r1x = strip.tile([P, NT, 5 + JP], F32)
            r22p = strip.tile([P, NT, 5], F32)
            cs = strip.tile([P, N], F32)  # exp(2*S12) partial column sums
            # symmetric passes: partial column sums for relative chunks d=1..3
            cs_sym = strip.tile([P, 3 * 1024], F32)

            def rhs_one(t, h, jp, tag_n):
                r = rhsp.tile([P, KO, 512], F8, tag="rhs", name=f"rhs_{tag_n}")
                nc.sync.dma_start(r[:], kp(ag_out[t, h][jp * D : (jp + 1) * D]))
                return r

            def sym_rhs(t, h, dlt, tag_n):
                # chunk ((cid+dlt)&7): register-offset row slice of the gather
                r = rhsp.tile([P, KO, 512], F8, tag="rhs", name=f"rhs_{tag_n}")
                sv = make_scalar_value(sync_regs[dlt - 1])
                nc.sync.dma_start(r[:], kp(ag_out[t, h][bass.ds(sv, D)]))
                return r

            def sim_iter(lhs, tt, rt0, rt1, accum, cs_sl=None):
                # fp8 DoubleRow: each matmul consumes a [P, 2, x] K-slab pair
                # (K=256); dots carry the NS^2 scale, undone in the exp scale.
                ps = psA.tile([P, 1024], F32, tag="ps_big", name="ps_sim")
                for kt in range(0, KO, 2):
                    for ch, rt in ((0, rt0), (1, rt1)):
                        nc.tensor.matmul(
                            ps[:, bass.ts(ch, 512)],
                            lhs[:, kt : kt + 2, bass.ts(tt, P)],
                            rt[:, kt : kt + 2, :],
                            start=(kt == 0),
                            stop=(kt == KO - 2),
                            perf_mode=DR,
                        )
                ex = expp.tile([P, 1024], F32, tag="ex")
                nc.scalar.activation(
                    ex[:], ps[:], AF.Exp, scale=2.0 / (NS * NS), accum_out=accum
                )
                if cs_sl is not None:
                    for ch in range(2):
                        if cs_sl[ch] is not None:
                            nc.vector.tensor_tensor(
                                cs_sl[ch], cs_sl[ch], ex[:, bass.ts(ch, 512)], ALU.add
                            )

            def colsum_flush(jp, h):
                # cs chunk (jp, h) complete -> bf16 stage, reduce over partitions,
                # ship to the ReduceScatter input at its global-j offset.
                g = jp * 1024 + h * 512
                csb = scratch.tile([P, 512], BF, tag="csb", bufs=2, name=f"csb{jp}_{h}")
                nc.vector.tensor_copy(csb[:], cs[:, bass.ds(g, 512)])
                cp = psB.tile([1, 512], F32, tag="ps_small", name=f"cp{jp}_{h}")
                nc.tensor.matmul(cp[:], ones_bf[:], csb[:], start=True, stop=True)
                cst = scratch.tile([1, 512], F32, tag="cst", bufs=2, name=f"cst{jp}_{h}")
                nc.vector.tensor_copy(cst[:], cp[:])
                nc.gpsimd.dma_start(rs_in[g : g + 512], cst[:])

            def sym_flush(dlt, rs_buf, tg):
                # colsums of chunk (cid+dlt) -> rs slice ((cid+dlt)&7)*1024
                for h in range(2):
                    base = (dlt - 1) * 1024 + h * 512
                    csb = scratch.tile([P, 512], BF, tag="csb", bufs=2, name=f"yb{tg}{dlt}{h}")
                    nc.vector.tensor_copy(csb[:], cs_sym[:, bass.ds(base, 512)])
                    cp = psB.tile([1, 512], F32, tag="ps_small", name=f"yp{tg}{dlt}{h}")
                    nc.tensor.matmul(cp[:], ones_bf[:], csb[:], start=True, stop=True)
                    cst = scratch.tile([1, 512], F32, tag="cst", bufs=2, name=f"yt{tg}{dlt}{h}")
                    nc.vector.tensor_copy(cst[:], cp[:])
                    svg = make_scalar_value(gp_regs[dlt - 1])
                    nc.gpsimd.dma_start(rs_buf[bass.ds(svg + h * 512, 512)], cst[:])

            def sim_pass(lhs, t, racc, is_s12, col_base=0):
                # phase h=0 runs entirely on the first AllGather half so it can
                # start before the second half lands; pair chunks share one exp.
                col = col_base
                for h in (0, 1):
                    for pj in range(0, JP, 2):
                        rt0 = rhs_one(t, h, pj, f"{t}{h}{pj}")
                        rt1 = rhs_one(t, h, pj + 1, f"{t}{h}{pj + 1}")
                        for tt in range(NT):
                            cs_sl = None
                            if is_s12:
                                cs_sl = (
                                    cs[:, bass.ds(pj * 1024 + h * 512, 512)],
                                    cs[:, bass.ds((pj + 1) * 1024 + h * 512, 512)],
                                )
                            sim_iter(lhs, tt, rt0, rt1, racc[:, tt, col : col + 1], cs_sl)
                        if is_s12:
                            colsum_flush(pj, h)
                            colsum_flush(pj + 1, h)
                        col += 1

            def sym_pass(lhs, t, racc, col_base, rs_buf, rs_out_buf, local_first):
                # S11/S22 with block symmetry: only relative chunks d=0..4.
                # d=0 streams straight from SBUF (no gather dependency), d=4 is
                # rowsum-only on both pair endpoints (computed twice, no
                # exchange), d=1..3 also accumulate column sums that a
                # ReduceScatter-add routes to the owning core's rows.
                nc.vector.memset(cs_sym[:], 0.0)
                local = (("L", 0), ("L", 1))
                # pass A wants the gather-free local pair first (fills the PE
                # while AG1a lands); B2 wants the exchanged chunks (d=1..3)
                # finished as early as possible so the colsum flushes and their
                # ReduceScatter overlap the remaining pairs instead of the tail
                if local_first:
                    sched = [local, ((1, 0), (2, 0)), ((3, 0), (4, 0)),
                             ((1, 1), (2, 1)), ((3, 1), (4, 1))]
                    flush_after = {3: (1, 2), 4: (3,)}
                else:
                    # one exchanged chunk completes per pair-iter, spreading the
                    # dynamic flush DMAs out so the ReduceScatter doorbell fires
                    # right after col 2 with ~2 pair-iters of compute left
                    sched = [((3, 0), (3, 1)), ((1, 0), (1, 1)), ((2, 0), (2, 1)),
                             ((4, 0), (4, 1)), local]
                    flush_after = {0: (3,), 1: (1,), 2: (2,)}
                for col, (s0, s1) in enumerate(sched):
                    rts, css = [], []
                    for d, h in (s0, s1):
                        if d == "L":
                            rts.append(lhs[:, :, bass.ds(h * 512, 512)])
                            css.append(None)
                        else:
                            rts.append(sym_rhs(t, h, d, f"y{t}{d}{h}"))
                            css.append(
                                cs_sym[:, bass.ds((d - 1) * 1024 + h * 512, 512)]
                                if d <= 3 else None
                            )
                    for tt in range(NT):
                        cs_sl = tuple(css) if any(c is not None for c in css) else None
                        sim_iter(lhs, tt, rts[0], rts[1],
                                 racc[:, tt, col_base + col : col_base + col + 1], cs_sl)
                    for dlt in flush_after.get(col, ()):
                        sym_flush(dlt, rs_buf, t)
                nc.gpsimd.collective_compute(
                    "ReduceScatter", ALU.add, replica_groups=rg,
                    ins=[rs_buf[:].opt()], outs=[rs_out_buf[:].opt()],
                )

            # cid rides the gpsimd queue: its host-dram latency (~30us) head-of-
            # line-blocks whatever queue it sits on, and gpsimd's early work
            # (the rs zero-fills) is already gated on the registers behind it
            cidt = small.tile([1, 1], mybir.dt.int32, tag="cidt")
            nc.gpsimd.dma_start(cidt[:], cid[:])
            # --- per-core id -> engine registers holding ((cid+d)&7)*1024 ---
            # (row offset into the flat gathered buffers / element offset into
            # the symmetric-colsum ReduceScatter inputs)
            sync_regs, gp_regs = [], []
            for eng, regs, nregs in ((nc.sync, sync_regs, 4), (nc.gpsimd, gp_regs, 3)):
                for dlt in range(1, nregs + 1):
                    reg = rstack.enter_context(eng.register(name=f"cd{dlt}"))
                    eng.reg_load(reg, cidt[:])
                    eng.reg_alu(reg, make_scalar_value(reg), dlt, ALU.add)
                    eng.reg_alu(reg, make_scalar_value(reg), 7, ALU.bitwise_and)
                    eng.reg_alu(reg, make_scalar_value(reg), D, ALU.mult)
                    regs.append(reg)
            # zero the symmetric RS inputs (gpsimd queue: FIFO-before the
            # dynamic colsum flushes that later fill 3 of the 8 slots)
            zsrc = small.tile([P, N // P], F32, tag="zsrc")
            nc.vector.memset(zsrc[:], 0.0)
            nc.gpsimd.dma_start(pt(rs11_in[:]), zsrc[:])
            nc.gpsimd.dma_start(pt(rs22_in[:]), zsrc[:])

            # ---------------- p_i = n1_i . n2_i (local diag of S12, x NS^2) ----------------
            pps = [psB.tile([1, 512], F32, name=f"pps{_c}", tag="ps_small") for _c in range(2)]
            for kt in range(KO):
                q = scratch.tile([P, BLK], BF, tag="sq")
                nc.vector.tensor_tensor(q[:], n1_f8[:, kt, :], n2_f8[:, kt, :], ALU.mult)
                for ch in range(2):
                    nc.tensor.matmul(
                        pps[ch][:],
                        ones_bf[:],
                        q[:, bass.ts(ch, 512)],
                        start=(kt == 0),
                        stop=(kt == KO - 1),
                    )
            for ch in range(2):
                p_c = small.tile([1, 512], F32, tag="p_c", name=f"p_c{ch}", bufs=2)
                nc.vector.tensor_copy(p_c[:], pps[ch][:])
                nc.gpsimd.dma_start(p_dram[ch * 512 : (ch + 1) * 512], p_c[:])
            # ---- pass A: S11 symmetric (lhs n1, rhs n1 local + gathered) ----
            sym_pass(n1_f8, 1, r1x, 0, rs11_in, rs11_out, local_first=True)

            # ---- pass B1: S12 (lhs n1, rhs gathered n2) + incremental colsums ----
            nc.vector.memset(cs[:], 0.0)
            sim_pass(n1_f8, 2, r1x, True, col_base=5)
            nc.gpsimd.collective_compute(
                "ReduceScatter", ALU.add, replica_groups=rg,
                ins=[rs_in[:].opt()], outs=[rs_out[:].opt()],
            )

            # ---- pass B2: S22 symmetric; both RS overlap it / its tail ----
            sym_pass(n2_f8, 2, r22p, 0, rs22_in, rs22_out, local_first=False)

            # ---------------- final loss:  0.5*ln(d1*d2) - 2*p/NS^2 ----------------
            c12 = small.tile([P, NT], F32, tag="c12")
            nc.sync.dma_start(c12[:], pt(rs_out[:]))
            c11 = small.tile([P, NT], F32, tag="c11")
            nc.sync.dma_start(c11[:], pt(rs11_out[:]))
            c22 = small.tile([P, NT], F32, tag="c22")
            nc.sync.dma_start(c22[:], pt(rs22_out[:]))
            p2 = small.tile([P, NT], F32, tag="p2")
            nc.sync.dma_start(p2[:], pt(p_dram[:]))
            pm = small.tile([P, NT], F32, tag="pm")
            nc.vector.tensor_scalar(pm[:], p2[:], -2.0 / (NS * NS), None, ALU.mult)

            d1 = small.tile([P, NT], F32, tag="d1")
            d2 = small.tile([P, NT], F32, tag="d2")
            nc.vector.reduce_sum(d1[:], r1x[:], axis=mybir.AxisListType.X)
            nc.vector.tensor_tensor(d1[:], d1[:], c11[:], ALU.add)
            nc.vector.tensor_scalar(d1[:], d1[:], -E2, None, ALU.add)
            nc.vector.reduce_sum(d2[:], r22p[:], axis=mybir.AxisListType.X)
            nc.vector.tensor_tensor(d2[:], d2[:], c12[:], ALU.add)
            nc.vector.tensor_tensor(d2[:], d2[:], c22[:], ALU.add)
            nc.vector.tensor_scalar(d2[:], d2[:], -E2, None, ALU.add)
            nc.vector.tensor_tensor(d1[:], d1[:], d2[:], ALU.mult)
            lns = small.tile([P, NT], F32, tag="lns")
            nc.scalar.activation(lns[:], d1[:], AF.Ln)
            loss = small.tile([P, NT], F32, tag="loss")
            nc.vector.tensor_scalar(loss[:], lns[:], 0.5, None, ALU.mult)
            nc.vector.tensor_tensor(loss[:], loss[:], pm[:], ALU.add)
            nc.sync.dma_start(pt(out[:]), loss[:])

    rstack.close()
    nc.finalize()
    return nc


@lru_cache(maxsize=1)
def _built():
    return _build()


def _kph(a):
    # [D, x] K-major -> on-chip [128, KO, x] (kp rearrange done on host)
    return np.ascontiguousarray(a.reshape(KO, P, -1).transpose(1, 0, 2))


def _pth(v):
    # [1024] -> on-chip [128, 8] (pt rearrange done on host)
    return np.ascontiguousarray(v.reshape(NT, P).T)


def _prep_inputs(z1, z2, fc1_w, fc1_b, fc2_w, fc2_b):
    f8 = ml_dtypes.float8_e4m3  # TRN FP8_EXP4-compatible below +-240
    w1t = _kph(np.ascontiguousarray(np.asarray(fc1_w, np.float32).T * WS).astype(f8))
    w2t = _kph(np.ascontiguousarray(np.asarray(fc2_w, np.float32).T * WS).astype(f8))
    b1 = _pth(np.asarray(fc1_b, np.float32))
    b2p = _pth(
        (np.asarray(fc2_b, np.float32) - np.asarray(fc2_w, np.float32).sum(axis=1)).astype(
            np.float32
        )
    )
    in_maps = []
    for c in range(NCORES):
        sl = slice(c * BLK, (c + 1) * BLK)
        in_maps.append(
            {
                "cid": np.array([c], np.int32),
                "z1t": _kph(np.ascontiguousarray(np.asarray(z1[sl], np.float32).T).astype(f8)),
                "z2t": _kph(np.ascontiguousarray(np.asarray(z2[sl], np.float32).T).astype(f8)),
                "w1t": w1t,
                "w2t": w2t,
                "b1": b1,
                "b2p": b2p,
            }
        )
    return in_maps


def _install_ntff_shim():
    """Register the axon NTFF profile hook (antenv.axon_hooks is absent in
    this image; rebuild it from trn_agent_boot's ctypes recipe)."""
    import sys
    import types

    if "antenv.axon_hooks" in sys.modules:
        return True
    try:
        import antenv
        from trn_agent_boot.trn_boot import _ntff_profile_via_ctypes

        hook = _ntff_profile_via_ctypes("/opt/axon/libaxon_pjrt.so")
        if hook is None:
            return False
        m = types.ModuleType("antenv.axon_hooks")
        m._hook = hook
        m.get_axon_ntff_profile_hook = lambda: m._hook
        m.set_axon_ntff_profile_hook = lambda h: setattr(m, "_hook", h)
        sys.modules["antenv.axon_hooks"] = m
        antenv.axon_hooks = m
        # artifact upload needs egress; neuter it for local profiling
        import concourse.bass_utils as _bu

        _bu.upload_artifacts = lambda tmpdir: f"file://{tmpdir}"
        return True
    except Exception as e:
        print(f"ntff shim unavailable: {e!r}")
        return False


def _run(in_maps, trace=False):
    nc = _built()
    if trace and not _install_ntff_shim():
        trace = False
    last = None
    for attempt in range(3):
        try:
            res = run_bass_kernel_spmd(nc, in_maps, list(range(NCORES)), trace=trace)
            if all(np.isfinite(res.results[c]["out"]).all() for c in range(NCORES)):
                return res
            print("nonfinite output, retrying")
        except Exception as e:  # device occasionally wedged from a prior process
            last = e
            if "UNRECOVERABLE" not in str(e) and "UNAVAILABLE" not in str(e):
                raise
            print(f"device error (attempt {attempt}): retrying")
    if last is not None:
        raise last
    return res


def kernel(z1, z2, fc1_w, fc1_b, fc2_w, fc2_b):
    in_maps = _prep_inputs(z1, z2, fc1_w, fc1_b, fc2_w, fc2_b)
    res = _run(in_maps, trace=os.environ.get("KERNEL_TRACE", "") == "1")
    if res.exec_time_ns is not None:
        print(f"HW exec time: {res.exec_time_ns} ns")
    out = np.concatenate([res.results[c]["out"] for c in range(NCORES)])
    return out.astype(np.float32)


# revision 80
# speedup vs baseline: 1.0128x; 1.0128x over previous
"""Trainium2 Bass kernel for nn_CLLayer (SimCLR-style contrastive loss).

Math (reference, tau=0.5):
    h1 = elu(z1 @ W1.T + b1) @ W2.T + b2 ; h2 likewise
    n1, n2 = row-normalized h1, h2
    l1_i = log(sum_j exp(2*n1_i.n1_j) + sum_j exp(2*n1_i.n2_j) - e^2) - 2*n1_i.n2_i
    l2_i = log(sum_j exp(2*n2_i.n2_j) + sum_j exp(2*n2_j.n1_i... ) - e^2) - 2*...
    out = 0.5*(l1+l2)

Sharding: row-parallel over N=8192 (1024 rows/core, 8 cores).
Each core: projects its row block, normalizes, AllGathers normalized
embeddings, computes its row-strip of the three distinct similarity
products (S12, S22, S11), exp+row-sums on the fly, column-sums of
exp(2*S12) via a ReduceScatter (between2 = between.T so l2's "between"
row sums are column sums of S12's exp).  Only 3 of 4 N^2*D products run.

All matmuls are fp8e4 with perf_mode=DoubleRow (2x PE throughput;
each MM consumes a [P, 2, x] K-slab pair, K=256).  fp8 subnormals are
avoided by x16 pre-scales: weights are scaled x16 on the host (undone
via the activation `scale`), normalized embeddings x16 on device
(undone in the exp scale 2/256 and the positive-pair term -2/256).
Each AllGather is split into two column halves so pass A can start on
the first half while the second is still in flight.

Host-side prep: transposes z blocks / weights to K-major (PE wants K on
partitions), casts matmul operands to fp8e4 (ml_dtypes.float8_e4m3
matches TRN FP8_EXP4 bit-exactly below 240), and folds the ELU "-1"
into an adjusted fc2 bias (b2' = b2 - fc2_w.sum(1)) so ELU is computed
as relu(x) + min(exp(x),1) without the subtract (device ELU' = elu+1).
"""

import math
import os
from contextlib import ExitStack
from functools import lru_cache

import ml_dtypes
import numpy as np

import concourse.bacc as bacc
import concourse.bass as bass
import concourse.mybir as mybir
import concourse.tile as tile
from concourse.bass import make_scalar_value
from concourse.bass_utils import run_bass_kernel_spmd

N, D = 8192, 1024
NCORES = 8
BLK = N // NCORES  # 1024
P = 128
KO = D // P  # 8 k-tiles
NT = BLK // P  # 8 i-tiles per core
JP = NCORES  # 8 j-chunks of 1024 (= core blocks)
E2 = float(np.exp(2.0))  # exp(1/tau), tau=0.5
BF = mybir.dt.bfloat16
F8 = mybir.dt.float8e4
F32 = mybir.dt.float32
NS = 16.0  # fp8 pre-scale on normalized embeddings
WS = 16.0  # fp8 pre-scale on weights (host side)
DR = mybir.MatmulPerfMode.DoubleRow
AF = mybir.ActivationFunctionType
ALU = mybir.AluOpType


def _build():
    nc = bacc.Bacc("TRN2", target_bir_lowering=False, debug=False, num_devices=NCORES)

    # inputs arrive pre-arranged in on-chip layouts ([P, KO, x] K-major, [P, KO]
    # bias) — strided reads from host dram are latency-bound (a 4KB pt()-gather
    # measured 29us), so all reshaping happens on the host
    cid = nc.dram_tensor("cid", [1], mybir.dt.int32, kind="ExternalInput")
    z1t = nc.dram_tensor("z1t", [P, KO, BLK], F8, kind="ExternalInput")
    z2t = nc.dram_tensor("z2t", [P, KO, BLK], F8, kind="ExternalInput")
    w1t = nc.dram_tensor("w1t", [P, KO, D], F8, kind="ExternalInput")
    w2t = nc.dram_tensor("w2t", [P, KO, D], F8, kind="ExternalInput")
    b1 = nc.dram_tensor("b1", [P, KO], F32, kind="ExternalInput")
    b2p = nc.dram_tensor("b2p", [P, KO], F32, kind="ExternalInput")
    out = nc.dram_tensor("out", [BLK], F32, kind="ExternalOutput")

    kp = lambda ap: ap.rearrange("(ko ki) x -> ki ko x", ki=P)  # K-major -> [128, KO, x]
    pt = lambda ap: ap.rearrange("(t p) -> p t", p=P)  # [1024] -> [128, 8]

    rstack = ExitStack()
    with tile.TileContext(nc) as tc:
        with (
            tc.tile_pool(name="consts", bufs=1) as consts,
            tc.tile_pool(name="mats", bufs=1) as mats,
            tc.tile_pool(name="strip", bufs=1) as strip,
            tc.tile_pool(name="scratch", bufs=2) as scratch,
            tc.tile_pool(name="rhs", bufs=6) as rhsp,
            tc.tile_pool(name="expp", bufs=2) as expp,
            tc.tile_pool(name="small", bufs=1) as small,
            tc.tile_pool(name="psA", bufs=3, space="PSUM") as psA,
            tc.tile_pool(name="psB", bufs=2, space="PSUM") as psB,
            tc.tile_pool(name="dram", bufs=1, space="DRAM") as dram,
        ):
            # ---------------- constants / inputs (proj1's needs first) ----------------
            w1_sb = consts.tile([P, KO, D], F8)
            w2_sb = consts.tile([P, KO, D], F8)
            b1_sb = consts.tile([P, KO], F32)
            b2_sb = consts.tile([P, KO], F32)
            z_sb = mats.tile([P, KO, BLK], F8, tag="zt")
            # halved transfers so proj1's first matmuls start sooner
            nc.sync.dma_start(w1_sb[:, :, 0:512], w1t[:, :, 0:512])
            nc.sync.dma_start(z_sb[:, :, 0:512], z1t[:, :, 0:512])
            nc.sync.dma_start(w1_sb[:, :, 512:1024], w1t[:, :, 512:1024])
            nc.sync.dma_start(z_sb[:, :, 512:1024], z1t[:, :, 512:1024])
            nc.sync.dma_start(b1_sb[:], b1[:])
            nc.sync.dma_start(w2_sb[:], w2t[:])
            nc.sync.dma_start(b2_sb[:], b2p[:])
            # own tag: staging must not pin rhs-pool slots (WAR on the pool
            # rotation would stall pass A's prefetch until proj2-ch1 retires)
            z2a = rhsp.tile([P, KO, 512], F8, tag="zstage", name="z2a", bufs=2)
            z2b = rhsp.tile([P, KO, 512], F8, tag="zstage", name="z2b", bufs=2)
            nc.sync.dma_start(z2a[:], z2t[:, :, 0:512])
            nc.sync.dma_start(z2b[:], z2t[:, :, 512:1024])
            ones_bf = consts.tile([P, 1], BF)
            nc.vector.memset(ones_bf[:], 1.0)
            lnns = consts.tile([1, 1], F32)
            nc.vector.memset(lnns[:], float(math.log(NS)))

            n1_sb = mats.tile([P, KO, BLK], BF, tag="n1")
            n2_sb = mats.tile([P, KO, BLK], BF, tag="n2")

            ag_in = {}
            ag_out = {}
            for t in (1, 2):
                for h in (0, 1):
                    ag_in[t, h] = dram.tile([D, 512], F8, name=f"ag{t}{h}_in")
                    ag_out[t, h] = dram.tile(
                        [NCORES * D, 512], F8, addr_space="Shared", name=f"ag{t}{h}_out"
                    )
            rs_in = dram.tile([N], F32)
            rs_out = dram.tile([BLK], F32)
            rs11_in = dram.tile([N], F32)
            rs11_out = dram.tile([BLK], F32)
            rs22_in = dram.tile([N], F32)
            rs22_out = dram.tile([BLK], F32)
            rn_dram = dram.tile([2, BLK], BF)
            p_dram = dram.tile([BLK], F32)

            rg = [list(range(NCORES))]


            # ------------ projection + normalize (into n_sb + n_f8), per tensor ------------
            # Column-half-outer: each 512-column half runs L1 -> L2 -> sumsq ->
            # rn -> fp8 cast -> its AllGather trigger before the other half
            # starts, so AG h=0 is in flight ~half a projection early and
            # collective-duration variance hides under the remaining compute.
            def proj_l1(z_at, elu_sb, ch):
                # layer 1: a1T[o, i] = W1T.T @ zT (K=d);
                # elu+1 = relu(y) + min(exp(y), 1), y = ps/WS + b1
                sl = bass.ds(ch * 512, 512)
                for ot in range(KO):
                    ps = psA.tile([P, 512], F32, tag="ps_big", name="ps_l1")
                    for kt in range(0, KO, 2):
                        nc.tensor.matmul(
                            ps[:],
                            w1_sb[:, kt : kt + 2, bass.ts(ot, P)],
                            z_at(kt, ch),
                            start=(kt == 0),
                            stop=(kt == KO - 2),
                            perf_mode=DR,
                        )
                    bcol = b1_sb[:, ot : ot + 1]
                    e_t = scratch.tile([P, 512], F32, tag="e_t")
                    r_t = scratch.tile([P, 512], F32, tag="r_t")
                    nc.scalar.activation(e_t[:], ps[:], AF.Exp, bias=bcol, scale=1.0 / WS)
                    nc.scalar.activation(r_t[:], ps[:], AF.Relu, bias=bcol, scale=1.0 / WS)
                    nc.vector.tensor_scalar(e_t[:], e_t[:], 1.0, None, ALU.min)
                    nc.vector.tensor_tensor(elu_sb[:, ot, sl], e_t[:], r_t[:], ALU.add)

            def proj_l2_tail(elu_sb, n_sb, n_f8, rn_slot, t, ch):
                sl = bass.ds(ch * 512, 512)
                # layer 2 -> n_sb (holds hT until scaled)
                ssps = psB.tile([1, 512], F32, name=f"ssps{t}{ch}", tag="ps_small")
                for ot in range(KO):
                    ps = psA.tile([P, 512], F32, tag="ps_big", name="ps_l2")
                    for kt in range(0, KO, 2):
                        nc.tensor.matmul(
                            ps[:],
                            w2_sb[:, kt : kt + 2, bass.ts(ot, P)],
                            elu_sb[:, kt : kt + 2, sl],
                            start=(kt == 0),
                            stop=(kt == KO - 2),
                            perf_mode=DR,
                        )
                    # bias-add stays on DVE: it is also what drains L2's PSUM,
                    # and the scalar queue backlogs during its fetch-stall
                    # recovery (an Identity-ACT variant measured ~20us slower)
                    nc.vector.tensor_scalar(
                        n_sb[:, ot, sl], ps[:], 1.0 / WS, b2_sb[:, ot : ot + 1],
                        ALU.mult, ALU.add,
                    )
                    # sumsq over d (partitions) via ones-matmul on h*h; the
                    # square runs on DVE — the scalar queue is the proj-region
                    # bottleneck (late instruction-fetch wake + ACT backlog) and
                    # this also avoids Square<->Exp ACT-table reloads
                    sq = scratch.tile([P, 512], BF, tag="sq")
                    nc.vector.tensor_tensor(sq[:], n_sb[:, ot, sl], n_sb[:, ot, sl], ALU.mult)
                    nc.tensor.matmul(
                        ssps[:], ones_bf[:], sq[:],
                        start=(ot == 0), stop=(ot == KO - 1),
                    )
                # rn = NS/||h||: rsqrt = NS*Exp(-0.5*Ln(s)) on the ACT
                # tables (DVE reciprocal is single-lane slow; the Ln/Exp
                # tables already bound the kernel's overall accuracy)
                l_c = small.tile([1, 512], F32, tag="l_c", name=f"l_c{t}{ch}", bufs=2)
                rn_c = small.tile([1, 512], BF, tag="rn_c", name=f"rn_c{t}{ch}", bufs=2)
                nc.scalar.activation(l_c[:], ssps[:], AF.Ln)
                nc.scalar.activation(rn_c[:], l_c[:], AF.Exp, scale=-0.5, bias=lnns[:])
                nc.scalar.dma_start(rn_dram[rn_slot : rn_slot + 1, sl], rn_c[:])
                rn_bc = scratch.tile([P, 512], BF, tag="rnbc", bufs=2, name=f"rn_bc{t}{ch}")
                nc.scalar.dma_start(
                    rn_bc[:],
                    rn_dram[rn_slot : rn_slot + 1, sl].to_broadcast((P, 512)),
                )
                for kt in range(KO):
                    nc.vector.tensor_tensor(
                        n_f8[:, kt, sl], n_sb[:, kt, sl], rn_bc[:], ALU.mult
                    )
                nc.scalar.dma_start(kp(ag_in[t, ch][:]), n_f8[:, :, sl])
                nc.gpsimd.collective_compute(
                    "AllGather", ALU.bypass, replica_groups=rg,
                    ins=[ag_in[t, ch][:].opt()], outs=[ag_out[t, ch][:].opt()],
                )

            elu1 = mats.tile([P, KO, BLK], F8, tag="elu")
            # own slots: n_f8 ch-0 writes must not WAR-wait on the elu/z slots'
            # ch-1 readers, or the early AllGather trigger serializes away
            n1_f8 = mats.tile([P, KO, BLK], F8, tag="n1f8", name="n1_f8")
            elu2 = mats.tile([P, KO, BLK], F8, tag="elu2", name="elu2")
            n2_f8 = mats.tile([P, KO, BLK], F8, tag="n2f8", name="n2_f8")
            z1_at = lambda kt, ch: z_sb[:, kt : kt + 2, bass.ds(ch * 512, 512)]
            z2_at = lambda kt, ch: (z2a if ch == 0 else z2b)[:, kt : kt + 2, :]
            # phase-sequential: each (tensor, half) runs L1 then L2+tail so its
            # AllGather triggers as early as possible (AG1a ~60us); the next
            # phase's L1 matmuls then cover the tail's activation drain
            for ch in (0, 1):
                proj_l1(z1_at, elu1, ch)
                proj_l2_tail(elu1, n1_sb, n1_f8, 0, 1, ch)
                proj_l1(z2_at, elu2, ch)
                proj_l2_tail(elu2, n2_sb, n2_f8, 1, 2, ch)


            # rowsum partials: S11 (5 symmetric pair-cols) and S12 (8) share one
            # tile so a single reduce yields r11_local + r12
            r1x = strip.tile([P, NT, 5 + JP], F32)
            r22p = strip.tile([P, NT, 5], F32)
            cs = strip.tile([P, N], F32)  # exp(2*S12) partial column sums
            # symmetric passes: partial column sums for relative chunks d=1..3
            cs_sym = strip.tile([P, 3 * 1024], F32)

            def rhs_one(t, h, jp, tag_n):
                r = rhsp.tile([P, KO, 512], F8, tag="rhs", name=f"rhs_{tag_n}")
                nc.sync.dma_start(r[:], kp(ag_out[t, h][jp * D : (jp + 1) * D]))
                return r

            def sym_rhs(t, h, dlt, tag_n):
                # chunk ((cid+dlt)&7): register-offset row slice of the gather
                r = rhsp.tile([P, KO, 512], F8, tag="rhs", name=f"rhs_{tag_n}")
                sv = make_scalar_value(sync_regs[dlt - 1])
                nc.sync.dma_start(r[:], kp(ag_out[t, h][bass.ds(sv, D)]))
                return r

            def sim_iter(lhs, tt, rt0, rt1, accum, cs_sl=None):
                # fp8 DoubleRow: each matmul consumes a [P, 2, x] K-slab pair
                # (K=256); dots carry the NS^2 scale, undone in the exp scale.
                ps = psA.tile([P, 1024], F32, tag="ps_big", name="ps_sim")
                for kt in range(0, KO, 2):
                    for ch, rt in ((0, rt0), (1, rt1)):
                        nc.tensor.matmul(
                            ps[:, bass.ts(ch, 512)],
                            lhs[:, kt : kt + 2, bass.ts(tt, P)],
                            rt[:, kt : kt + 2, :],
                            start=(kt == 0),
                            stop=(kt == KO - 2),
                            perf_mode=DR,
                        )
                ex = expp.tile([P, 1024], F32, tag="ex")
                nc.scalar.activation(
                    ex[:], ps[:], AF.Exp, scale=2.0 / (NS * NS), accum_out=accum
                )
                if cs_sl is not None:
                    for ch in range(2):
                        if cs_sl[ch] is not None:
                            nc.vector.tensor_tensor(
                                cs_sl[ch], cs_sl[ch], ex[:, bass.ts(ch, 512)], ALU.add
                            )

            def colsum_flush(jp, h):
                # cs chunk (jp, h) complete -> bf16 stage, reduce over partitions,
                # ship to the ReduceScatter input at its global-j offset.
                g = jp * 1024 + h * 512
                csb = scratch.tile([P, 512], BF, tag="csb", bufs=2, name=f"csb{jp}_{h}")
                nc.vector.tensor_copy(csb[:], cs[:, bass.ds(g, 512)])
                cp = psB.tile([1, 512], F32, tag="ps_small", name=f"cp{jp}_{h}")
                nc.tensor.matmul(cp[:], ones_bf[:], csb[:], start=True, stop=True)
                cst = scratch.tile([1, 512], F32, tag="cst", bufs=2, name=f"cst{jp}_{h}")
                nc.vector.tensor_copy(cst[:], cp[:])
                nc.gpsimd.dma_start(rs_in[g : g + 512], cst[:])

            def sym_flush(dlt, rs_buf, tg):
                # colsums of chunk (cid+dlt) -> rs slice ((cid+dlt)&7)*1024
                for h in range(2):
                    base = (dlt - 1) * 1024 + h * 512
                    csb = scratch.tile([P, 512], BF, tag="csb", bufs=2, name=f"yb{tg}{dlt}{h}")
                    nc.vector.tensor_copy(csb[:], cs_sym[:, bass.ds(base, 512)])
                    cp = psB.tile([1, 512], F32, tag="ps_small", name=f"yp{tg}{dlt}{h}")
                    nc.tensor.matmul(cp[:], ones_bf[:], csb[:], start=True, stop=True)
                    cst = scratch.tile([1, 512], F32, tag="cst", bufs=2, name=f"yt{tg}{dlt}{h}")
                    nc.vector.tensor_copy(cst[:], cp[:])
                    svg = make_scalar_value(gp_regs[dlt - 1])
                    nc.gpsimd.dma_start(rs_buf[bass.ds(svg + h * 512, 512)], cst[:])

            def sim_pass(lhs, t, racc, is_s12, col_base=0):
                # phase h=0 runs entirely on the first AllGather half so it can
                # start before the second half lands; pair chunks share one exp.
                col = col_base
                for h in (0, 1):
                    for pj in range(0, JP, 2):
                        rt0 = rhs_one(t, h, pj, f"{t}{h}{pj}")
                        rt1 = rhs_one(t, h, pj + 1, f"{t}{h}{pj + 1}")
                        for tt in range(NT):
                            cs_sl = None
                            if is_s12:
                                cs_sl = (
                                    cs[:, bass.ds(pj * 1024 + h * 512, 512)],
                                    cs[:, bass.ds((pj + 1) * 1024 + h * 512, 512)],
                                )
                            sim_iter(lhs, tt, rt0, rt1, racc[:, tt, col : col + 1], cs_sl)
                        if is_s12:
                            colsum_flush(pj, h)
                            colsum_flush(pj + 1, h)
                        col += 1

            def sym_pass(lhs, t, racc, col_base, rs_buf, rs_out_buf, local_first):
                # S11/S22 with block symmetry: only relative chunks d=0..4.
                # d=0 streams straight from SBUF (no gather dependency), d=4 is
                # rowsum-only on both pair endpoints (computed twice, no
                # exchange), d=1..3 also accumulate column sums that a
                # ReduceScatter-add routes to the owning core's rows.
                nc.vector.memset(cs_sym[:], 0.0)
                local = (("L", 0), ("L", 1))
                # pass A wants the gather-free local pair first (fills the PE
                # while AG1a lands); B2 wants the exchanged chunks (d=1..3)
                # finished as early as possible so the colsum flushes and their
                # ReduceScatter overlap the remaining pairs instead of the tail
                if local_first:
                    sched = [local, ((1, 0), (2, 0)), ((3, 0), (4, 0)),
                             ((1, 1), (2, 1)), ((3, 1), (4, 1))]
                    flush_after = {3: (1, 2), 4: (3,)}
                else:
                    # one exchanged chunk completes per pair-iter, spreading the
                    # dynamic flush DMAs out so the ReduceScatter doorbell fires
                    # right after col 2 with ~2 pair-iters of compute left
                    sched = [((3, 0), (3, 1)), ((1, 0), (1, 1)), ((2, 0), (2, 1)),
                             ((4, 0), (4, 1)), local]
                    flush_after = {0: (3,), 1: (1,), 2: (2,)}
                for col, (s0, s1) in enumerate(sched):
                    rts, css = [], []
                    for d, h in (s0, s1):
                        if d == "L":
                            rts.append(lhs[:, :, bass.ds(h * 512, 512)])
                            css.append(None)
                        else:
                            rts.append(sym_rhs(t, h, d, f"y{t}{d}{h}"))
                            css.append(
                                cs_sym[:, bass.ds((d - 1) * 1024 + h * 512, 512)]
                                if d <= 3 else None
                            )
                    for tt in range(NT):
                        cs_sl = tuple(css) if any(c is not None for c in css) else None
                        sim_iter(lhs, tt, rts[0], rts[1],
                                 racc[:, tt, col_base + col : col_base + col + 1], cs_sl)
                    for dlt in flush_after.get(col, ()):
                        sym_flush(dlt, rs_buf, t)
                nc.gpsimd.collective_compute(
                    "ReduceScatter", ALU.add, replica_groups=rg,
                    ins=[rs_buf[:].opt()], outs=[rs_out_buf[:].opt()],
                )

            # cid rides the gpsimd queue: its host-dram latency (~30us) head-of-
            # line-blocks whatever queue it sits on, and gpsimd's early work
            # (the rs zero-fills) is already gated on the registers behind it
            cidt = small.tile([1, 1], mybir.dt.int32, tag="cidt")
            nc.gpsimd.dma_start(cidt[:], cid[:])
            # --- per-core id -> engine registers holding ((cid+d)&7)*1024 ---
            # (row offset into the flat gathered buffers / element offset into
            # the symmetric-colsum ReduceScatter inputs)
            sync_regs, gp_regs = [], []
            for eng, regs, nregs in ((nc.sync, sync_regs, 4), (nc.gpsimd, gp_regs, 3)):
                for dlt in range(1, nregs + 1):
                    reg = rstack.enter_context(eng.register(name=f"cd{dlt}"))
                    eng.reg_load(reg, cidt[:])
                    eng.reg_alu(reg, make_scalar_value(reg), dlt, ALU.add)
                    eng.reg_alu(reg, make_scalar_value(reg), 7, ALU.bitwise_and)
                    eng.reg_alu(reg, make_scalar_value(reg), D, ALU.mult)
                    regs.append(reg)
            # zero the symmetric RS inputs (gpsimd queue: FIFO-before the
            # dynamic colsum flushes that later fill 3 of the 8 slots)
            zsrc = small.tile([P, N // P], F32, tag="zsrc")
            nc.vector.memset(zsrc[:], 0.0)
            nc.gpsimd.dma_start(pt(rs11_in[:]), zsrc[:])
            nc.gpsimd.dma_start(pt(rs22_in[:]), zsrc[:])

            # ---------------- p_i = n1_i . n2_i (local diag of S12, x NS^2) ----------------
            pps = [psB.tile([1, 512], F32, name=f"pps{_c}", tag="ps_small") for _c in range(2)]
            for kt in range(KO):
                q = scratch.tile([P, BLK], BF, tag="sq")
                nc.vector.tensor_tensor(q[:], n1_f8[:, kt, :], n2_f8[:, kt, :], ALU.mult)
                for ch in range(2):
                    nc.tensor.matmul(
                        pps[ch][:],
                        ones_bf[:],
                        q[:, bass.ts(ch, 512)],
                        start=(kt == 0),
                        stop=(kt == KO - 1),
                    )
            for ch in range(2):
                p_c = small.tile([1, 512], F32, tag="p_c", name=f"p_c{ch}", bufs=2)
                nc.vector.tensor_copy(p_c[:], pps[ch][:])
                nc.gpsimd.dma_start(p_dram[ch * 512 : (ch + 1) * 512], p_c[:])
            # ---- pass A: S11 symmetric (lhs n1, rhs n1 local + gathered) ----
            sym_pass(n1_f8, 1, r1x, 0, rs11_in, rs11_out, local_first=True)

            # ---- pass B1: S12 (lhs n1, rhs gathered n2) + incremental colsums ----
            nc.vector.memset(cs[:], 0.0)
            sim_pass(n1_f8, 2, r1x, True, col_base=5)
            nc.gpsimd.collective_compute(
                "ReduceScatter", ALU.add, replica_groups=rg,
                ins=[rs_in[:].opt()], outs=[rs_out[:].opt()],
            )

            # ---- pass B2: S22 symmetric; both RS overlap it / its tail ----
            sym_pass(n2_f8, 2, r22p, 0, rs22_in, rs22_out, local_first=False)

            # ---------------- final loss:  0.5*ln(d1*d2) - 2*p/NS^2 ----------------
            c12 = small.tile([P, NT], F32, tag="c12")
            nc.sync.dma_start(c12[:], pt(rs_out[:]))
            c11 = small.tile([P, NT], F32, tag="c11")
            nc.sync.dma_start(c11[:], pt(rs11_out[:]))
            c22 = small.tile([P, NT], F32, tag="c22")
            nc.sync.dma_start(c22[:], pt(rs22_out[:]))
            p2 = small.tile([P, NT], F32, tag="p2")
            nc.sync.dma_start(p2[:], pt(p_dram[:]))
            pm = small.tile([P, NT], F32, tag="pm")
            nc.vector.tensor_scalar(pm[:], p2[:], -2.0 / (NS * NS), None, ALU.mult)

            d1 = small.tile([P, NT], F32, tag="d1")
            d2 = small.tile([P, NT], F32, tag="d2")
            nc.vector.reduce_sum(d1[:], r1x[:], axis=mybir.AxisListType.X)
            nc.vector.tensor_tensor(d1[:], d1[:], c11[:], ALU.add)
            nc.vector.tensor_scalar(d1[:], d1[:], -E2, None, ALU.add)
            nc.vector.reduce_sum(d2[:], r22p[:], axis=mybir.AxisListType.X)
            nc.vector.tensor_tensor(d2[:], d2[:], c12[:], ALU.add)
            nc.vector.tensor_tensor(d2[:], d2[:], c22[:], ALU.add)
            nc.vector.tensor_scalar(d2[:], d2[:], -E2, None, ALU.add)
            nc.vector.tensor_tensor(d1[:], d1[:], d2[:], ALU.mult)
            lns = small.tile([P, NT], F32, tag="lns")
            nc.scalar.activation(lns[:], d1[:], AF.Ln)
            loss = small.tile([P, NT], F32, tag="loss")
            nc.vector.tensor_scalar(loss[:], lns[:], 0.5, None, ALU.mult)
            nc.vector.tensor_tensor(loss[:], loss[:], pm[:], ALU.add)
            nc.sync.dma_start(pt(out[:]), loss[:])

    rstack.close()
    nc.finalize()
    return nc


@lru_cache(maxsize=1)
def _built():
    return _build()


def _kph(a):
    # [D, x] K-major -> on-chip [128, KO, x] (kp rearrange done on host)
    return np.ascontiguousarray(a.reshape(KO, P, -1).transpose(1, 0, 2))


def _pth(v):
    # [1024] -> on-chip [128, 8] (pt rearrange done on host)
    return np.ascontiguousarray(v.reshape(NT, P).T)


def _prep_inputs(z1, z2, fc1_w, fc1_b, fc2_w, fc2_b):
    f8 = ml_dtypes.float8_e4m3  # TRN FP8_EXP4-compatible below +-240
    w1t = _kph(np.ascontiguousarray(np.asarray(fc1_w, np.float32).T * WS).astype(f8))
    w2t = _kph(np.ascontiguousarray(np.asarray(fc2_w, np.float32).T * WS).astype(f8))
    b1 = _pth(np.asarray(fc1_b, np.float32))
    b2p = _pth(
        (np.asarray(fc2_b, np.float32) - np.asarray(fc2_w, np.float32).sum(axis=1)).astype(
            np.float32
        )
    )
    in_maps = []
    for c in range(NCORES):
        sl = slice(c * BLK, (c + 1) * BLK)
        in_maps.append(
            {
                "cid": np.array([c], np.int32),
                "z1t": _kph(np.ascontiguousarray(np.asarray(z1[sl], np.float32).T).astype(f8)),
                "z2t": _kph(np.ascontiguousarray(np.asarray(z2[sl], np.float32).T).astype(f8)),
                "w1t": w1t,
                "w2t": w2t,
                "b1": b1,
                "b2p": b2p,
            }
        )
    return in_maps


def _install_ntff_shim():
    """Register the axon NTFF profile hook (antenv.axon_hooks is absent in
    this image; rebuild it from trn_agent_boot's ctypes recipe)."""
    import sys
    import types

    if "antenv.axon_hooks" in sys.modules:
        return True
    try:
        import antenv
        from trn_agent_boot.trn_boot import _ntff_profile_via_ctypes

        hook = _ntff_profile_via_ctypes("/opt/axon/libaxon_pjrt.so")
        if hook is None:
            return False
        m = types.ModuleType("antenv.axon_hooks")
        m._hook = hook
        m.get_axon_ntff_profile_hook = lambda: m._hook
        m.set_axon_ntff_profile_hook = lambda h: setattr(m, "_hook", h)
        sys.modules["antenv.axon_hooks"] = m
        antenv.axon_hooks = m
        # artifact upload needs egress; neuter it for local profiling
        import concourse.bass_utils as _bu

        _bu.upload_artifacts = lambda tmpdir: f"file://{tmpdir}"
        return True
    except Exception as e:
        print(f"ntff shim unavailable: {e!r}")
        return False


def _run(in_maps, trace=False):
    nc = _built()
    if trace and not _install_ntff_shim():
        trace = False
    last = None
    for attempt in range(3):
        try:
            res = run_bass_kernel_spmd(nc, in_maps, list(range(NCORES)), trace=trace)
            if all(np.isfinite(res.results[c]["out"]).all() for c in range(NCORES)):
                return res
            print("nonfinite output, retrying")
        except Exception as e:  # device occasionally wedged from a prior process
            last = e
            if "UNRECOVERABLE" not in str(e) and "UNAVAILABLE" not in str(e):
                raise
            print(f"device error (attempt {attempt}): retrying")
    if last is not None:
        raise last
    return res


def kernel(z1, z2, fc1_w, fc1_b, fc2_w, fc2_b):
    in_maps = _prep_inputs(z1, z2, fc1_w, fc1_b, fc2_w, fc2_b)
    res = _run(in_maps, trace=os.environ.get("KERNEL_TRACE", "") == "1")
    if res.exec_time_ns is not None:
        print(f"HW exec time: {res.exec_time_ns} ns")
    out = np.concatenate([res.results[c]["out"] for c in range(NCORES)])
    return out.astype(np.float32)
